# revision 10
# baseline (speedup 1.0000x reference)
"""AttnDecoderRNN Trainium2 kernel: 8-core SPMD.

Sharding:
  - Attention g-dim, ctx h-dim, GRU gate n-dim: column-sharded 128/core.
  - Per step: AllReduce(partial scores), AllGather(ctx^T slice), AllGather(h^T slice).
  - Output projection: vocab-sharded 4000/core; batched (B*T, H) @ W_out^T after the
    recurrence with a distributed log-softmax (AllReduce max, AllReduce sumexp).
Host does embedding gather, transposes/slices, and final concatenation only.
"""
import sys
import numpy as np

sys.path.insert(0, "/opt/trn_rl_repo")

import ml_dtypes
from concourse import bass, bacc, mybir
from concourse import tile
from concourse.bass_utils import run_bass_kernel_spmd

bf16 = ml_dtypes.bfloat16
F32 = mybir.dt.float32
BF16 = mybir.dt.bfloat16
AF = mybir.ActivationFunctionType
ALU = mybir.AluOpType

B, S, T, H, V = 64, 128, 32, 1024, 32000
NC = 8
HC = H // NC          # 128  per-core hidden/g slice
GC = 3 * H // NC      # 384  per-core gate rows
VC = V // NC          # 4000 per-core vocab slice
KC = H // 128         # 8    K-chunks
BS = B * S            # 8192
TB = T * B            # 2048
SOS = 1
RG = [list(range(NC))]

_prog_cache = {}


def _build_program():
    if "nc" in _prog_cache:
        return _prog_cache["nc"]
    nc = bacc.Bacc("TRN2", target_bir_lowering=False, debug=False, num_devices=8)

    # ---------------- inputs (per-core content) ----------------
    encT = nc.declare_dram_parameter("encT", [H, BS], BF16, isOutput=False)
    UaT = nc.declare_dram_parameter("UaT", [H, HC], BF16, isOutput=False)
    WaT = nc.declare_dram_parameter("WaT", [H, HC], BF16, isOutput=False)
    kbias = nc.declare_dram_parameter("kbias", [HC, 1], F32, isOutput=False)
    Vaw = nc.declare_dram_parameter("Vaw", [HC, 1], BF16, isOutput=False)
    embT = nc.declare_dram_parameter("embT", [H, TB], BF16, isOutput=False)
    Wih1T = nc.declare_dram_parameter("Wih1T", [H, GC], BF16, isOutput=False)
    Wih2T = nc.declare_dram_parameter("Wih2T", [H, GC], BF16, isOutput=False)
    WhhT = nc.declare_dram_parameter("WhhT", [H, GC], BF16, isOutput=False)
    gbias = nc.declare_dram_parameter("gbias", [1, GC], BF16, isOutput=False)
    hbias = nc.declare_dram_parameter("hbias", [1, GC], BF16, isOutput=False)
    enc_hs = nc.declare_dram_parameter("enc_hs", [S, B, HC], BF16, isOutput=False)
    h0T = nc.declare_dram_parameter("h0T", [128, KC, B], BF16, isOutput=False)
    h0nat = nc.declare_dram_parameter("h0nat", [B, HC], F32, isOutput=False)
    WoutT = nc.declare_dram_parameter("WoutT", [H, VC], BF16, isOutput=False)
    bout = nc.declare_dram_parameter("bout", [1, VC], BF16, isOutput=False)
    ident = nc.declare_dram_parameter("ident", [128, 128], F32, isOutput=False)

    lp_part = nc.declare_dram_parameter("lp_part", [B, T, VC], F32, isOutput=True)
    attn = nc.declare_dram_parameter("attn", [B, T, S], F32, isOutput=True)
    hout = nc.declare_dram_parameter("hout", [B, HC], F32, isOutput=True)

    with tile.TileContext(nc) as tc:
        with (
            tc.tile_pool(name="per", bufs=1) as per,
            tc.tile_pool(name="dram", bufs=2, space="DRAM") as dram,
        ):
            # -------- persistent tiles --------
            keys_t = per.tile([128, B, S], BF16, tag="keys")       # (g, b, s)
            gi_emb = per.tile([B, T, GC], F32, tag="giemb")        # emb@Wih1T per t
            enc_hs_t = per.tile([S, B, HC], BF16, tag="enchs")
            WaT_t = per.tile([128, KC, HC], BF16, tag="WaT")
            WhhT_t = per.tile([128, KC, GC], BF16, tag="WhhT")
            Wih2T_t = per.tile([128, KC, GC], BF16, tag="Wih2T")
            Va_t = per.tile([HC, 1], BF16, tag="Va")
            kbias_t = per.tile([HC, 1], F32, tag="kbias")
            gbias_t = per.tile([1, GC], BF16, tag="gbias")
            hbias_t = per.tile([1, GC], BF16, tag="hbias")
            ident_t = per.tile([128, 128], F32, tag="ident")
            ones64 = per.tile([1, B], BF16, tag="ones64")
            ones128 = per.tile([1, 128], BF16, tag="ones128")
            bout_t = per.tile([1, VC], BF16, tag="bout")
            hT = per.tile([128, KC, B], BF16, tag="hT")            # full h^T
            hnat = per.tile([B, HC], F32, tag="hnat")              # own h slice
            ctxTf = per.tile([128, KC, B], BF16, tag="ctxTf")      # full ctx^T
            H_hist = per.tile([128, KC, T, B], BF16, tag="Hh")     # h^T history

            nc.vector.memset(ones64[:], 1.0)
            nc.vector.memset(ones128[:], 1.0)
            nc.sync.dma_start(WaT_t[:], WaT[:].rearrange("(kc p) n -> p kc n", p=128))
            nc.sync.dma_start(WhhT_t[:], WhhT[:].rearrange("(kc p) n -> p kc n", p=128))
            nc.sync.dma_start(Wih2T_t[:], Wih2T[:].rearrange("(kc p) n -> p kc n", p=128))
            nc.sync.dma_start(Va_t[:], Vaw[:])
            nc.sync.dma_start(kbias_t[:], kbias[:])
            nc.sync.dma_start(gbias_t[:], gbias[:])
            nc.sync.dma_start(hbias_t[:], hbias[:])
            nc.sync.dma_start(ident_t[:], ident[:])
            nc.sync.dma_start(bout_t[:], bout[:])
            nc.sync.dma_start(enc_hs_t[:], enc_hs[:])
            nc.sync.dma_start(hT[:], h0T[:])
            nc.sync.dma_start(hnat[:], h0nat[:])

            # -------- phase A1: gi_emb = emb @ Wih1T (per t) --------
            with (
                tc.tile_pool(name="bootA", bufs=1) as bootA,
                tc.tile_pool(name="psA", bufs=2, space="PSUM") as psA,
            ):
                embT_t = bootA.tile([128, KC, TB], BF16, tag="embT")
                Wih1T_t = bootA.tile([128, KC, GC], BF16, tag="Wih1T")
                nc.sync.dma_start(embT_t[:], embT[:].rearrange("(kc p) n -> p kc n", p=128))
                nc.sync.dma_start(Wih1T_t[:], Wih1T[:].rearrange("(kc p) n -> p kc n", p=128))
                for t_ in range(T):
                    gp = psA.tile([B, GC], F32, tag="gp")
                    for kc in range(KC):
                        nc.tensor.matmul(
                            gp[:], embT_t[:, kc, t_ * B:(t_ + 1) * B],
                            Wih1T_t[:, kc, :], start=(kc == 0), stop=(kc == KC - 1))
                    nc.vector.tensor_copy(gi_emb[:, t_, :], gp[:])

            # -------- phase A2: keys = (enc @ Ua^T)^T + (ba+bu) --------
            with (
                tc.tile_pool(name="bootB", bufs=2) as bootB,
                tc.tile_pool(name="psB", bufs=2, space="PSUM") as psB,
            ):
                UaT_t = bootB.tile([128, KC, HC], BF16, tag="UaT")
                nc.sync.dma_start(UaT_t[:], UaT[:].rearrange("(kc p) n -> p kc n", p=128))
                keys_flat = keys_t[:].rearrange("p b s -> p (b s)")
                for blk in range(4):
                    encb = bootB.tile([128, KC, 2048], BF16, tag="encb")
                    nc.sync.dma_start(
                        encb[:],
                        encT[:].rearrange("(kc p) n -> p kc n", p=128)[:, :, blk * 2048:(blk + 1) * 2048])
                    for nbl in range(4):
                        kp = psB.tile([128, 512], F32, tag="kp")
                        for kc in range(KC):
                            nc.tensor.matmul(
                                kp[:], UaT_t[:, kc, :],
                                encb[:, kc, nbl * 512:(nbl + 1) * 512],
                                start=(kc == 0), stop=(kc == KC - 1))
                        nc.vector.tensor_scalar(
                            keys_flat[:, blk * 2048 + nbl * 512: blk * 2048 + (nbl + 1) * 512],
                            kp[:], kbias_t[:], None, op0=ALU.add)

            # -------- phase B: recurrence --------
            _st_cm = tc.tile_pool(name="st", bufs=2)
            _ps_cm = tc.tile_pool(name="psmain", bufs=2, space="PSUM")
            st = _st_cm.__enter__()
            ps = _ps_cm.__enter__()
            for t in range(T):
                # q^T = (Wa_c h)^T  -> (128g, 64b)
                q_ps = ps.tile([128, B], F32, tag="p64")
                for kc in range(KC):
                    nc.tensor.matmul(q_ps[:], WaT_t[:, kc, :], hT[:, kc, :],
                                     start=(kc == 0), stop=(kc == KC - 1))
                qT_bf = st.tile([128, B], BF16, tag="qT")
                nc.vector.tensor_copy(qT_bf[:], q_ps[:])

                # add + tanh + Va matvecs, chunked over b (16 b per chunk)
                sc_ps = ps.tile([128, 2048], F32, tag="scp", bufs=1)
                for ch in range(4):
                    bsl = slice(ch * 16, (ch + 1) * 16)
                    add_t = st.tile([128, 16, S], BF16, tag="addt")
                    nc.vector.tensor_tensor(
                        add_t[:], keys_t[:, bsl, :],
                        qT_bf[:, bsl].unsqueeze(2).broadcast_to([128, 16, S]),
                        op=ALU.add)
                    tanh_t = st.tile([128, 16, S], BF16, tag="tanht")
                    nc.scalar.activation(tanh_t[:], add_t[:], AF.Tanh)
                    for bl in range(16):
                        b = ch * 16 + bl
                        j, k = b % 4, b // 4
                        nc.tensor.matmul(
                            sc_ps[32 * j:32 * j + 1, k * S:(k + 1) * S],
                            Va_t[:], tanh_t[:, bl, :], start=True, stop=True,
                            tile_position=(0, 32 * j))
                scp_s = st.tile([128, 2048], F32, tag="scps")
                nc.vector.tensor_copy(scp_s[:], sc_ps[:])
                sc_in = dram.tile([4 * 2048], F32, tag="scin")
                # b-major bounce: idx = (k*4+j)*S + s ; source rows 0,32,64,96
                nc.sync.dma_start(
                    sc_in[:].rearrange("(k j s) -> j k s", j=4, s=S),
                    scp_s[:].rearrange("(j r) (k s) -> j r k s", r=32, s=S)[:, 0, :, :])
                sc_out = dram.tile([NC * 4 * 2048], F32, tag="scout")
                nc.gpsimd.collective_compute(
                    "AllGather", ALU.bypass, replica_groups=RG,
                    ins=[sc_in.opt()], outs=[sc_out.opt()])
                scg = st.tile([B, NC, S], F32, tag="scg")
                nc.sync.dma_start(
                    scg[:], sc_out[:].rearrange("(c b s) -> b c s", s=S, c=NC))
                sc_t = st.tile([B, S], F32, tag="sct")
                nc.vector.tensor_tensor(scg[:, 0, :], scg[:, 0, :], scg[:, 4, :], op=ALU.add)
                nc.vector.tensor_tensor(scg[:, 1, :], scg[:, 1, :], scg[:, 5, :], op=ALU.add)
                nc.vector.tensor_tensor(scg[:, 2, :], scg[:, 2, :], scg[:, 6, :], op=ALU.add)
                nc.vector.tensor_tensor(scg[:, 3, :], scg[:, 3, :], scg[:, 7, :], op=ALU.add)
                nc.vector.tensor_tensor(scg[:, 0, :], scg[:, 0, :], scg[:, 2, :], op=ALU.add)
                nc.vector.tensor_tensor(scg[:, 1, :], scg[:, 1, :], scg[:, 3, :], op=ALU.add)
                nc.vector.tensor_tensor(sc_t[:], scg[:, 0, :], scg[:, 1, :], op=ALU.add)

                # softmax -> w (64, 128)
                negmax = st.tile([B, 1], F32, tag="negmax")
                nc.vector.tensor_reduce(negmax[:], sc_t[:], axis=mybir.AxisListType.X,
                                        op=ALU.max, negate=True)
                w_t = st.tile([B, S], F32, tag="wt")
                sume = st.tile([B, 1], F32, tag="sume")
                nc.scalar.activation(w_t[:], sc_t[:], AF.Exp, bias=negmax[:],
                                     accum_out=sume[:])
                recip = st.tile([B, 1], F32, tag="recip")
                nc.vector.reciprocal(recip[:], sume[:])
                nc.vector.tensor_scalar(w_t[:], w_t[:], recip[:], None, op0=ALU.mult)
                nc.sync.dma_start(attn[:, t, :], w_t[:])

                # w^T (128s, 64b) bf16 via PE transpose
                tr_ps = ps.tile([S, B], F32, tag="p64")
                nc.tensor.matmul(tr_ps[:], w_t[:], ident_t[0:64, 0:64], is_transpose=True)
                wT_bf = st.tile([S, B], BF16, tag="wTb")
                nc.vector.tensor_copy(wT_bf[:], tr_ps[:])

                # ctx^T slice (128h, 64b): per-b matvec enc_hs[b]^T w[b]
                ctx_ps = ps.tile([HC, B], F32, tag="p64")
                for b in range(B):
                    nc.tensor.matmul(ctx_ps[:, b:b + 1], enc_hs_t[:, b, :],
                                     wT_bf[:, b:b + 1], start=True, stop=True)
                ctxT_bf = st.tile([HC, B], BF16, tag="ctxTb")
                nc.vector.tensor_copy(ctxT_bf[:], ctx_ps[:])
                ctx_in = dram.tile([HC * B], BF16, tag="ctxin")
                nc.sync.dma_start(ctx_in[:].rearrange("(p f) -> p f", p=HC), ctxT_bf[:])
                ctx_out = dram.tile([NC * HC * B], BF16, tag="ctxout")
                nc.gpsimd.collective_compute(
                    "AllGather", ALU.bypass, replica_groups=RG,
                    ins=[ctx_in.opt()], outs=[ctx_out.opt()])
                for cc in range(NC):
                    nc.sync.dma_start(
                        ctxTf[:, cc, :],
                        ctx_out[cc * HC * B:(cc + 1) * HC * B].rearrange("(p f) -> p f", p=HC))

                # gates
                gi_ps = ps.tile([B, GC], F32, tag="p384")
                for kc in range(KC):
                    nc.tensor.matmul(gi_ps[:], ctxTf[:, kc, :], Wih2T_t[:, kc, :],
                                     start=(kc == 0), stop=False)
                nc.tensor.matmul(gi_ps[:], ones64[:], gbias_t[:], start=False, stop=True)
                gh_ps = ps.tile([B, GC], F32, tag="p384")
                for kc in range(KC):
                    nc.tensor.matmul(gh_ps[:], hT[:, kc, :], WhhT_t[:, kc, :],
                                     start=(kc == 0), stop=False)
                nc.tensor.matmul(gh_ps[:], ones64[:], hbias_t[:], start=False, stop=True)
                gihe = st.tile([B, GC], F32, tag="gihe")
                nc.vector.scalar_tensor_tensor(
                    gihe[:], gi_ps[:], 1.0, gi_emb[:, t, :], op0=ALU.mult, op1=ALU.add)
                rzsum = st.tile([B, 256], F32, tag="rzsum")
                nc.vector.tensor_tensor(rzsum[:], gihe[:, 0:256], gh_ps[:, 0:256], op=ALU.add)
                tz = st.tile([B, 256], F32, tag="tz")
                nc.scalar.activation(tz[:], rzsum[:], AF.Tanh, scale=0.5)
                # r*hn*2 = (tz_r + 1) * hn   [r = 0.5 + 0.5 tz_r]
                rhn2 = st.tile([B, HC], F32, tag="rhn2")
                nc.vector.scalar_tensor_tensor(
                    rhn2[:], tz[:, 0:128], 1.0, gh_ps[:, 256:384],
                    op0=ALU.add, op1=ALU.mult)
                npre = st.tile([B, HC], F32, tag="npre")
                nc.vector.scalar_tensor_tensor(
                    npre[:], rhn2[:], 0.5, gihe[:, 256:384], op0=ALU.mult, op1=ALU.add)
                n_t = st.tile([B, HC], F32, tag="nt")
                nc.scalar.activation(n_t[:], npre[:], AF.Tanh)
                # h_new = n + z*(h-n), z = 0.5 + 0.5 tz_z:
                hmn = st.tile([B, HC], F32, tag="hmn")
                nc.vector.tensor_tensor(hmn[:], hnat[:], n_t[:], op=ALU.subtract)
                zm2 = st.tile([B, HC], F32, tag="zm2")
                nc.vector.scalar_tensor_tensor(
                    zm2[:], tz[:, 128:256], 1.0, hmn[:], op0=ALU.add, op1=ALU.mult)
                nc.vector.scalar_tensor_tensor(
                    hnat[:], zm2[:], 0.5, n_t[:], op0=ALU.mult, op1=ALU.add)

                # h exchange: transpose own slice, AllGather, rebuild hT
                tr2_ps = ps.tile([HC, B], F32, tag="p64")
                nc.tensor.matmul(tr2_ps[:], hnat[:], ident_t[0:64, 0:64], is_transpose=True)
                hT_my = st.tile([HC, B], BF16, tag="hTmy")
                nc.vector.tensor_copy(hT_my[:], tr2_ps[:])
                h_in = dram.tile([HC * B], BF16, tag="hin")
                nc.sync.dma_start(h_in[:].rearrange("(p f) -> p f", p=HC), hT_my[:])
                h_out = dram.tile([NC * HC * B], BF16, tag="hout2")
                nc.gpsimd.collective_compute(
                    "AllGather", ALU.bypass, replica_groups=RG,
                    ins=[h_in.opt()], outs=[h_out.opt()])
                for cc in range(NC):
                    nc.sync.dma_start(
                        hT[:, cc, :],
                        h_out[cc * HC * B:(cc + 1) * HC * B].rearrange("(p f) -> p f", p=HC))
                nc.vector.tensor_copy(H_hist[:, :, t, :], hT[:])

            nc.sync.dma_start(hout[:], hnat[:])
            _ps_cm.__exit__(None, None, None)
            _st_cm.__exit__(None, None, None)
            # -------- phase C: logits + log-softmax --------
            with (
                tc.tile_pool(name="pc", bufs=2) as pc,
                tc.tile_pool(name="pc1", bufs=1) as pc1,
                tc.tile_pool(name="psC", bufs=2, space="PSUM") as psC,
            ):
                logits_dram = dram.tile([128, 16, VC], BF16, tag="logits")
                rmax = pc1.tile([128, 16], F32, tag="rmax")
                sume_c = pc1.tile([128, 16], F32, tag="sumec")
                NB = 8
                NW = VC // NB  # 500
                for nb in range(NB):
                    wtile = pc.tile([128, KC, NW], BF16, tag="wt2")
                    nc.sync.dma_start(
                        wtile[:],
                        WoutT[:].rearrange("(kc p) v -> p kc v", p=128)[:, :, nb * NW:(nb + 1) * NW])
                    for m in range(16):
                        lp_ps = psC.tile([128, NW], F32, tag="lpp")
                        for kc in range(KC):
                            nc.tensor.matmul(
                                lp_ps[:], H_hist[:, kc, 2 * m:2 * m + 2, :],
                                wtile[:, kc, :], start=(kc == 0), stop=False)
                        nc.tensor.matmul(lp_ps[:], ones128[:],
                                         bout_t[:, nb * NW:(nb + 1) * NW],
                                         start=False, stop=True)
                        lsb = pc.tile([128, NW], BF16, tag="lsb")
                        nc.scalar.copy(lsb[:], lp_ps[:])
                        nc.sync.dma_start(logits_dram[:, m, nb * NW:(nb + 1) * NW], lsb[:])
                        tmpm = pc.tile([128, 1], F32, tag="tmpm")
                        nc.vector.tensor_reduce(tmpm[:], lsb[:],
                                                axis=mybir.AxisListType.X, op=ALU.max)
                        if nb == 0:
                            nc.vector.tensor_copy(rmax[:, m:m + 1], tmpm[:])
                        else:
                            nc.vector.tensor_tensor(rmax[:, m:m + 1], rmax[:, m:m + 1],
                                                    tmpm[:], op=ALU.max)
                mx_in = dram.tile([128 * 16], F32, tag="mxin")
                nc.sync.dma_start(mx_in[:].rearrange("(p f) -> p f", p=128), rmax[:])
                mx_out = dram.tile([128 * 16], F32, tag="mxout")
                nc.gpsimd.collective_compute(
                    "AllReduce", ALU.max, replica_groups=RG,
                    ins=[mx_in.opt()], outs=[mx_out.opt()])
                gmax = pc1.tile([128, 16], F32, tag="gmax")
                nc.sync.dma_start(gmax[:], mx_out[:].rearrange("(p f) -> p f", p=128))
                gneg = pc1.tile([128, 16], F32, tag="gneg")
                nc.vector.tensor_scalar(gneg[:], gmax[:], -1.0, None, op0=ALU.mult)

                for m in range(16):
                    lt = pc.tile([128, VC], BF16, tag="lt")
                    nc.sync.dma_start(lt[:], logits_dram[:, m, :])
                    esc = pc.tile([128, VC], BF16, tag="esc")
                    nc.scalar.activation(esc[:], lt[:], AF.Exp, bias=gneg[:, m:m + 1],
                                         accum_out=sume_c[:, m:m + 1])
                se_in = dram.tile([128 * 16], F32, tag="sein")
                nc.sync.dma_start(se_in[:].rearrange("(p f) -> p f", p=128), sume_c[:])
                se_out = dram.tile([128 * 16], F32, tag="seout")
                nc.gpsimd.collective_compute(
                    "AllReduce", ALU.add, replica_groups=RG,
                    ins=[se_in.opt()], outs=[se_out.opt()])
                gsum = pc1.tile([128, 16], F32, tag="gsum")
                nc.sync.dma_start(gsum[:], se_out[:].rearrange("(p f) -> p f", p=128))
                lnz = pc1.tile([128, 16], F32, tag="lnz")
                nc.scalar.activation(lnz[:], gsum[:], AF.Ln)
                nneg = pc1.tile([128, 16], F32, tag="nneg")
                nc.vector.tensor_tensor(nneg[:], gmax[:], lnz[:], op=ALU.add)
                nc.vector.tensor_scalar(nneg[:], nneg[:], -1.0, None, op0=ALU.mult)

                for m in range(16):
                    lt = pc.tile([128, VC], BF16, tag="lt")
                    nc.sync.dma_start(lt[:], logits_dram[:, m, :])
                    for h2 in range(2):
                        of = pc.tile([128, VC // 2], F32, tag="of")
                        nc.vector.tensor_scalar(
                            of[:], lt[:, h2 * (VC // 2):(h2 + 1) * (VC // 2)],
                            nneg[:, m:m + 1], None, op0=ALU.add)
                        for tt in range(2):
                            nc.sync.dma_start(
                                lp_part[:, 2 * m + tt,
                                        h2 * (VC // 2):(h2 + 1) * (VC // 2)],
                                of[64 * tt:64 * (tt + 1), :])

    nc.compile()
    _prog_cache["nc"] = nc
    return nc


def _host_prep(inputs):
    f = lambda x: np.asarray(x)
    enc = f(inputs["encoder_outputs"]).astype(np.float32)          # (B,S,H)
    h0 = f(inputs["encoder_hidden"]).astype(np.float32)[0]         # (B,H)
    tgt = f(inputs["target_tensor"])
    embedding = f(inputs["embedding"]).astype(np.float32)
    Wa, ba = f(inputs["Wa"]).astype(np.float32), f(inputs["ba"]).astype(np.float32)
    Ua, bu = f(inputs["Ua"]).astype(np.float32), f(inputs["bu"]).astype(np.float32)
    Va = f(inputs["Va"]).astype(np.float32)
    W_ih, b_ih = f(inputs["W_ih"]).astype(np.float32), f(inputs["b_ih"]).astype(np.float32)
    W_hh, b_hh = f(inputs["W_hh"]).astype(np.float32), f(inputs["b_hh"]).astype(np.float32)
    W_out, b_out = f(inputs["W_out"]).astype(np.float32), f(inputs["b_out"]).astype(np.float32)

    tokens = np.concatenate([np.full((B, 1), SOS, tgt.dtype), tgt[:, :-1]], axis=1)
    emb = embedding[tokens]                                        # (B,T,H)

    encT = np.ascontiguousarray(enc.transpose(2, 0, 1).reshape(H, BS)).astype(bf16)
    embT = np.ascontiguousarray(emb.transpose(2, 1, 0).reshape(H, TB)).astype(bf16)
    h0T = np.ascontiguousarray(
        h0.T.reshape(KC, 128, B).transpose(1, 0, 2)).astype(bf16)  # (128,kc,b)

    in_maps = []
    for c in range(NC):
        gs = slice(c * HC, (c + 1) * HC)
        rows = np.r_[c * HC:(c + 1) * HC, H + c * HC:H + (c + 1) * HC,
                     2 * H + c * HC:2 * H + (c + 1) * HC]
        vs = slice(c * VC, (c + 1) * VC)
        in_maps.append({
            "encT": encT,
            "UaT": np.ascontiguousarray(Ua[gs, :].T).astype(bf16),
            "WaT": np.ascontiguousarray(Wa[gs, :].T).astype(bf16),
            "kbias": (ba + bu)[gs].reshape(HC, 1).astype(np.float32),
            "Vaw": Va[0, gs].reshape(HC, 1).astype(bf16),
            "embT": embT,
            "Wih1T": np.ascontiguousarray(W_ih[rows, :H].T).astype(bf16),
            "Wih2T": np.ascontiguousarray(W_ih[rows, H:].T).astype(bf16),
            "WhhT": np.ascontiguousarray(W_hh[rows, :].T).astype(bf16),
            "gbias": (b_ih[rows] + np.concatenate(
                [b_hh[c * HC:(c + 1) * HC], b_hh[H + c * HC:H + (c + 1) * HC],
                 np.zeros(HC, np.float32)])).reshape(1, GC).astype(bf16),
            "hbias": np.concatenate(
                [np.zeros(2 * HC, np.float32),
                 b_hh[2 * H + c * HC:2 * H + (c + 1) * HC]]).reshape(1, GC).astype(bf16),
            "enc_hs": np.ascontiguousarray(enc[:, :, gs].transpose(1, 0, 2)).astype(bf16),
            "h0T": h0T,
            "h0nat": np.ascontiguousarray(h0[:, gs]).astype(np.float32),
            "WoutT": np.ascontiguousarray(W_out[vs, :].T).astype(bf16),
            "bout": b_out[vs].reshape(1, VC).astype(bf16),
            "ident": np.eye(128, dtype=np.float32),
        })
    return in_maps


def _run(inputs, trace=False):
    nc = _build_program()
    in_maps = _host_prep(inputs)
    res = run_bass_kernel_spmd(nc, in_maps, list(range(NC)), trace=trace)
    r = res.results
    lp = np.concatenate([r[c]["lp_part"] for c in range(NC)], axis=2)
    h_final = np.concatenate([r[c]["hout"] for c in range(NC)], axis=1)[None]
    attentions = r[0]["attn"]
    return (lp, h_final, attentions), res


def kernel(**inputs):
    out, _ = _run(inputs, trace=False)
    return out


def bench(inputs, iters=3):
    """Time device execution with pre-staged inputs (excludes host transfers)."""
    import time as _time
    import jax
    from jax.sharding import Mesh, PartitionSpec
    from jax.experimental.shard_map import shard_map
    from concourse import bass2jax as b2j
    from concourse.bass2jax import _bass_exec_p, partition_id_tensor
    import concourse.mybir as _mybir

    nc = _build_program()
    in_maps = _host_prep(inputs)
    n_cores = NC
    b2j.install_neuronx_cc_hook()
    partition_name = nc.partition_id_tensor.name if nc.partition_id_tensor else None
    in_names, out_names, out_avals, zero_outs = [], [], [], []
    for alloc in nc.m.functions[0].allocations:
        if not isinstance(alloc, _mybir.MemoryLocationSet):
            continue
        name = alloc.memorylocations[0].name
        if alloc.kind == "ExternalInput":
            if name != partition_name:
                in_names.append(name)
        elif alloc.kind == "ExternalOutput":
            import numpy as _np
            dt = _mybir.dt.np(alloc.dtype)
            out_names.append(name)
            out_avals.append(jax.core.ShapedArray(tuple(alloc.tensor_shape), dt))
            zero_outs.append(_np.zeros(tuple(alloc.tensor_shape), dt))
    n_params = len(in_names)
    n_outs = len(out_avals)
    in_names = in_names + out_names + ([partition_name] if partition_name else [])
    donate = tuple(range(n_params, n_params + n_outs))

    def _body(*args):
        operands = list(args)
        if partition_name is not None:
            operands.append(partition_id_tensor())
        return tuple(_bass_exec_p.bind(
            *operands, out_avals=tuple(out_avals), in_names=tuple(in_names),
            out_names=tuple(out_names), lowering_input_output_aliases=(),
            sim_require_finite=True, sim_require_nnan=True, nc=nc))

    devices = jax.devices()[:n_cores]
    mesh = Mesh(np.asarray(devices), ("core",))
    sharded = jax.jit(
        shard_map(_body, mesh=mesh,
                  in_specs=(PartitionSpec("core"),) * (n_params + n_outs),
                  out_specs=(PartitionSpec("core"),) * n_outs,
                  check_rep=False),
        donate_argnums=donate, keep_unused=True)
    per_core = [[np.asarray(m[nm]) for nm in in_names[:n_params]] for m in in_maps]
    concat_in = [np.concatenate([per_core[c][i] for c in range(n_cores)], axis=0)
                 for i in range(n_params)]
    from jax.sharding import NamedSharding
    shard = NamedSharding(mesh, PartitionSpec("core"))
    dev_in = [jax.device_put(x, shard) for x in concat_in]
    times = []
    out = None
    for it in range(iters + 1):
        zs = [jax.device_put(np.zeros((n_cores * z.shape[0], *z.shape[1:]), z.dtype), shard)
              for z in zero_outs]
        jax.block_until_ready(zs)
        t0 = _time.perf_counter()
        out = sharded(*dev_in, *zs)
        jax.block_until_ready(out)
        t1 = _time.perf_counter()
        times.append(t1 - t0)
    return times, out_names, out


# revision 14
# speedup vs baseline: 1.8145x; 1.8145x over previous
"""AttnDecoderRNN Trainium2 kernel: 8-core SPMD.

Sharding:
  - Attention g-dim, ctx h-dim, GRU gate n-dim: column-sharded 128/core.
  - Per step: AllReduce(partial scores), AllGather(ctx^T slice), AllGather(h^T slice).
  - Output projection: vocab-sharded 4000/core; batched (B*T, H) @ W_out^T after the
    recurrence with a distributed log-softmax (AllReduce max, AllReduce sumexp).
Host does embedding gather, transposes/slices, and final concatenation only.
"""
import sys
import numpy as np

sys.path.insert(0, "/opt/trn_rl_repo")

import ml_dtypes
from concourse import bass, bacc, mybir
from concourse import tile
from concourse.bass_utils import run_bass_kernel_spmd

bf16 = ml_dtypes.bfloat16
F32 = mybir.dt.float32
BF16 = mybir.dt.bfloat16
AF = mybir.ActivationFunctionType
ALU = mybir.AluOpType

B, S, T, H, V = 64, 128, 32, 1024, 32000
NC = 8
HC = H // NC          # 128  per-core hidden/g slice
GC = 3 * H // NC      # 384  per-core gate rows
VC = V // NC          # 4000 per-core vocab slice
KC = H // 128         # 8    K-chunks
BS = B * S            # 8192
TB = T * B            # 2048
SOS = 1
RG = [list(range(NC))]

_prog_cache = {}


def _build_program():
    if "nc" in _prog_cache:
        return _prog_cache["nc"]
    nc = bacc.Bacc("TRN2", target_bir_lowering=False, debug=False, num_devices=8)

    # ---------------- inputs (per-core content) ----------------
    encT = nc.declare_dram_parameter("encT", [H, BS], BF16, isOutput=False)
    UaT = nc.declare_dram_parameter("UaT", [H, HC], BF16, isOutput=False)
    WaT = nc.declare_dram_parameter("WaT", [H, HC], BF16, isOutput=False)
    kbias = nc.declare_dram_parameter("kbias", [HC, 1], F32, isOutput=False)
    Vaw = nc.declare_dram_parameter("Vaw", [HC, 1], BF16, isOutput=False)
    embT = nc.declare_dram_parameter("embT", [H, TB], BF16, isOutput=False)
    Wih1T = nc.declare_dram_parameter("Wih1T", [H, GC], BF16, isOutput=False)
    Wih2T = nc.declare_dram_parameter("Wih2T", [H, GC], BF16, isOutput=False)
    WhhT = nc.declare_dram_parameter("WhhT", [H, GC], BF16, isOutput=False)
    gbias = nc.declare_dram_parameter("gbias", [1, GC], BF16, isOutput=False)
    hbias = nc.declare_dram_parameter("hbias", [1, GC], BF16, isOutput=False)
    enc_hs = nc.declare_dram_parameter("enc_hs", [S, B, HC], BF16, isOutput=False)
    h0T = nc.declare_dram_parameter("h0T", [128, KC, B], BF16, isOutput=False)
    h0nat = nc.declare_dram_parameter("h0nat", [B, HC], F32, isOutput=False)
    WoutT = nc.declare_dram_parameter("WoutT", [H, VC], BF16, isOutput=False)
    bout = nc.declare_dram_parameter("bout", [1, VC], BF16, isOutput=False)
    ident = nc.declare_dram_parameter("ident", [128, 128], F32, isOutput=False)

    lp_part = nc.declare_dram_parameter("lp_part", [B, T, VC], F32, isOutput=True)
    attn = nc.declare_dram_parameter("attn", [B, T, S], F32, isOutput=True)
    hout = nc.declare_dram_parameter("hout", [B, HC], F32, isOutput=True)

    with tile.TileContext(nc) as tc:
        with (
            tc.tile_pool(name="per", bufs=1) as per,
            tc.tile_pool(name="dram", bufs=2, space="DRAM") as dram,
        ):
            # -------- persistent tiles --------
            keys_t = per.tile([128, B, S], BF16, tag="keys")       # (g, b, s)
            gi_emb = per.tile([B, T, GC], F32, tag="giemb")        # emb@Wih1T per t
            enc_hs_t = per.tile([S, B, HC], BF16, tag="enchs")
            WaT_t = per.tile([128, KC, HC], BF16, tag="WaT")
            WhhT_t = per.tile([128, KC, GC], BF16, tag="WhhT")
            Wih2T_t = per.tile([128, KC, GC], BF16, tag="Wih2T")
            Va_t = per.tile([HC, 1], BF16, tag="Va")
            kbias_t = per.tile([HC, 1], F32, tag="kbias")
            gbias_t = per.tile([1, GC], BF16, tag="gbias")
            hbias_t = per.tile([1, GC], BF16, tag="hbias")
            ident_t = per.tile([128, 128], F32, tag="ident")
            ones64 = per.tile([1, B], BF16, tag="ones64")
            ones128 = per.tile([1, 128], BF16, tag="ones128")
            bout_t = per.tile([1, VC], BF16, tag="bout")
            hT = per.tile([128, KC, B], BF16, tag="hT")            # full h^T
            hnat = per.tile([B, HC], F32, tag="hnat")              # own h slice
            ctxTf = per.tile([128, KC, B], BF16, tag="ctxTf")      # full ctx^T
            H_hist = per.tile([128, KC, T, B], BF16, tag="Hh")     # h^T history

            nc.vector.memset(ones64[:], 1.0)
            nc.vector.memset(ones128[:], 1.0)
            nc.sync.dma_start(WaT_t[:], WaT[:].rearrange("(kc p) n -> p kc n", p=128))
            nc.sync.dma_start(WhhT_t[:], WhhT[:].rearrange("(kc p) n -> p kc n", p=128))
            nc.sync.dma_start(Wih2T_t[:], Wih2T[:].rearrange("(kc p) n -> p kc n", p=128))
            nc.sync.dma_start(Va_t[:], Vaw[:])
            nc.sync.dma_start(kbias_t[:], kbias[:])
            nc.sync.dma_start(gbias_t[:], gbias[:])
            nc.sync.dma_start(hbias_t[:], hbias[:])
            nc.sync.dma_start(ident_t[:], ident[:])
            nc.sync.dma_start(bout_t[:], bout[:])
            nc.sync.dma_start(enc_hs_t[:], enc_hs[:])
            nc.sync.dma_start(hT[:], h0T[:])
            nc.sync.dma_start(hnat[:], h0nat[:])
            logits_dram = dram.tile([128, 16, VC], BF16, tag="logits")

            # -------- phase A1: gi_emb = emb @ Wih1T (per t) --------
            with (
                tc.tile_pool(name="bootA", bufs=1) as bootA,
                tc.tile_pool(name="psA", bufs=2, space="PSUM") as psA,
            ):
                embT_t = bootA.tile([128, KC, TB], BF16, tag="embT")
                Wih1T_t = bootA.tile([128, KC, GC], BF16, tag="Wih1T")
                nc.sync.dma_start(embT_t[:], embT[:].rearrange("(kc p) n -> p kc n", p=128))
                nc.sync.dma_start(Wih1T_t[:], Wih1T[:].rearrange("(kc p) n -> p kc n", p=128))
                for t_ in range(T):
                    gp = psA.tile([B, GC], F32, tag="gp")
                    for kc in range(KC):
                        nc.tensor.matmul(
                            gp[:], embT_t[:, kc, t_ * B:(t_ + 1) * B],
                            Wih1T_t[:, kc, :], start=(kc == 0), stop=(kc == KC - 1))
                    nc.vector.tensor_copy(gi_emb[:, t_, :], gp[:])

            # -------- phase A2: keys = (enc @ Ua^T)^T + (ba+bu) --------
            with (
                tc.tile_pool(name="bootB", bufs=2) as bootB,
                tc.tile_pool(name="psB", bufs=2, space="PSUM") as psB,
            ):
                UaT_t = bootB.tile([128, KC, HC], BF16, tag="UaT")
                nc.sync.dma_start(UaT_t[:], UaT[:].rearrange("(kc p) n -> p kc n", p=128))
                keys_flat = keys_t[:].rearrange("p b s -> p (b s)")
                for blk in range(4):
                    encb = bootB.tile([128, KC, 2048], BF16, tag="encb")
                    nc.sync.dma_start(
                        encb[:],
                        encT[:].rearrange("(kc p) n -> p kc n", p=128)[:, :, blk * 2048:(blk + 1) * 2048])
                    for nbl in range(4):
                        kp = psB.tile([128, 512], F32, tag="kp")
                        for kc in range(KC):
                            nc.tensor.matmul(
                                kp[:], UaT_t[:, kc, :],
                                encb[:, kc, nbl * 512:(nbl + 1) * 512],
                                start=(kc == 0), stop=(kc == KC - 1))
                        nc.vector.tensor_scalar(
                            keys_flat[:, blk * 2048 + nbl * 512: blk * 2048 + (nbl + 1) * 512],
                            kp[:], kbias_t[:], None, op0=ALU.add)

            # -------- phase B: recurrence --------
            _perB_cm = tc.tile_pool(name="perB", bufs=1)
            perB = _perB_cm.__enter__()
            WoutA = perB.tile([128, KC, VC // 4], BF16, tag="WoutA")
            rmax = perB.tile([128, 16], F32, tag="rmax")
            nc.sync.dma_start(
                WoutA[:],
                WoutT[:].rearrange("(kc p) v -> p kc v", p=128)[:, :, 0:VC // 4])
            _st_cm = tc.tile_pool(name="st", bufs=2)
            _ps_cm = tc.tile_pool(name="psmain", bufs=2, space="PSUM")
            st = _st_cm.__enter__()
            ps = _ps_cm.__enter__()
            for t in range(T):
                # q^T = (Wa_c h)^T  -> (128g, 64b)
                q_ps = ps.tile([128, B], F32, tag="p64")
                for kc in range(KC):
                    nc.tensor.matmul(q_ps[:], WaT_t[:, kc, :], hT[:, kc, :],
                                     start=(kc == 0), stop=(kc == KC - 1))
                qT_bf = st.tile([128, B], BF16, tag="qT")
                nc.vector.tensor_copy(qT_bf[:], q_ps[:])

                # add + tanh + Va matvecs, chunked over b (16 b per chunk)
                sc_in = dram.tile([4 * 2048], F32, tag="scin")
                for rnd in range(2):
                    sc_ps = ps.tile([128, 1024], F32, tag="scp", bufs=1)
                    for ch in range(2):
                        bsl = slice(rnd * 32 + ch * 16, rnd * 32 + (ch + 1) * 16)
                        add_t = st.tile([128, 16, S], BF16, tag="addt")
                        nc.vector.tensor_tensor(
                            add_t[:], keys_t[:, bsl, :],
                            qT_bf[:, bsl].unsqueeze(2).broadcast_to([128, 16, S]),
                            op=ALU.add)
                        tanh_t = st.tile([128, 16, S], BF16, tag="tanht")
                        nc.scalar.activation(tanh_t[:], add_t[:], AF.Tanh)
                        for bl in range(16):
                            b = rnd * 32 + ch * 16 + bl
                            j, k = b % 4, (b // 4) % 8
                            nc.tensor.matmul(
                                sc_ps[32 * j:32 * j + 1, k * S:(k + 1) * S],
                                Va_t[:], tanh_t[:, bl, :], start=True, stop=True,
                                tile_position=(0, 32 * j))
                    scp_s = st.tile([128, 1024], F32, tag="scps")
                    nc.vector.tensor_copy(scp_s[:], sc_ps[:])
                    # b-major bounce: idx = (k*4+j)*S + s within this round's half
                    nc.sync.dma_start(
                        sc_in[rnd * 4096:(rnd + 1) * 4096].rearrange(
                            "(k j s) -> j k s", j=4, s=S),
                        scp_s[:].rearrange("(j r) (k s) -> j r k s", r=32, s=S)[:, 0, :, :])
                sc_out = dram.tile([NC * 4 * 2048], F32, tag="scout")
                nc.gpsimd.collective_compute(
                    "AllGather", ALU.bypass, replica_groups=RG,
                    ins=[sc_in.opt()], outs=[sc_out.opt()])
                scg = st.tile([B, NC, S], F32, tag="scg")
                nc.sync.dma_start(
                    scg[:], sc_out[:].rearrange("(c b s) -> b c s", s=S, c=NC))
                sc_t = st.tile([B, S], F32, tag="sct")
                nc.vector.tensor_tensor(scg[:, 0, :], scg[:, 0, :], scg[:, 4, :], op=ALU.add)
                nc.vector.tensor_tensor(scg[:, 1, :], scg[:, 1, :], scg[:, 5, :], op=ALU.add)
                nc.vector.tensor_tensor(scg[:, 2, :], scg[:, 2, :], scg[:, 6, :], op=ALU.add)
                nc.vector.tensor_tensor(scg[:, 3, :], scg[:, 3, :], scg[:, 7, :], op=ALU.add)
                nc.vector.tensor_tensor(scg[:, 0, :], scg[:, 0, :], scg[:, 2, :], op=ALU.add)
                nc.vector.tensor_tensor(scg[:, 1, :], scg[:, 1, :], scg[:, 3, :], op=ALU.add)
                nc.vector.tensor_tensor(sc_t[:], scg[:, 0, :], scg[:, 1, :], op=ALU.add)

                # softmax -> w (64, 128)
                negmax = st.tile([B, 1], F32, tag="negmax")
                nc.vector.tensor_reduce(negmax[:], sc_t[:], axis=mybir.AxisListType.X,
                                        op=ALU.max, negate=True)
                w_t = st.tile([B, S], F32, tag="wt")
                sume = st.tile([B, 1], F32, tag="sume")
                nc.scalar.activation(w_t[:], sc_t[:], AF.Exp, bias=negmax[:],
                                     accum_out=sume[:])
                recip = st.tile([B, 1], F32, tag="recip")
                nc.vector.reciprocal(recip[:], sume[:])
                nc.vector.tensor_scalar(w_t[:], w_t[:], recip[:], None, op0=ALU.mult)
                nc.sync.dma_start(attn[:, t, :], w_t[:])

                # w^T (128s, 64b) bf16 via PE transpose
                tr_ps = ps.tile([S, B], F32, tag="p64")
                nc.tensor.matmul(tr_ps[:], w_t[:], ident_t[0:64, 0:64], is_transpose=True)
                wT_bf = st.tile([S, B], BF16, tag="wTb")
                nc.vector.tensor_copy(wT_bf[:], tr_ps[:])

                # ctx^T slice (128h, 64b): per-b matvec enc_hs[b]^T w[b]
                ctx_ps = ps.tile([HC, B], F32, tag="p64")
                for b in range(B):
                    nc.tensor.matmul(ctx_ps[:, b:b + 1], enc_hs_t[:, b, :],
                                     wT_bf[:, b:b + 1], start=True, stop=True)
                ctxT_bf = st.tile([HC, B], BF16, tag="ctxTb")
                nc.vector.tensor_copy(ctxT_bf[:], ctx_ps[:])
                ctx_in = dram.tile([HC * B], BF16, tag="ctxin")
                nc.sync.dma_start(ctx_in[:].rearrange("(p f) -> p f", p=HC), ctxT_bf[:])
                ctx_out = dram.tile([NC * HC * B], BF16, tag="ctxout")
                nc.gpsimd.collective_compute(
                    "AllGather", ALU.bypass, replica_groups=RG,
                    ins=[ctx_in.opt()], outs=[ctx_out.opt()])
                for cc in range(NC):
                    nc.sync.dma_start(
                        ctxTf[:, cc, :],
                        ctx_out[cc * HC * B:(cc + 1) * HC * B].rearrange("(p f) -> p f", p=HC))

                # gates
                gi_ps = ps.tile([B, GC], F32, tag="p384")
                for kc in range(KC):
                    nc.tensor.matmul(gi_ps[:], ctxTf[:, kc, :], Wih2T_t[:, kc, :],
                                     start=(kc == 0), stop=False)
                nc.tensor.matmul(gi_ps[:], ones64[:], gbias_t[:], start=False, stop=True)
                gh_ps = ps.tile([B, GC], F32, tag="p384")
                for kc in range(KC):
                    nc.tensor.matmul(gh_ps[:], hT[:, kc, :], WhhT_t[:, kc, :],
                                     start=(kc == 0), stop=False)
                nc.tensor.matmul(gh_ps[:], ones64[:], hbias_t[:], start=False, stop=True)
                gihe = st.tile([B, GC], F32, tag="gihe")
                nc.vector.scalar_tensor_tensor(
                    gihe[:], gi_ps[:], 1.0, gi_emb[:, t, :], op0=ALU.mult, op1=ALU.add)
                rzsum = st.tile([B, 256], F32, tag="rzsum")
                nc.vector.tensor_tensor(rzsum[:], gihe[:, 0:256], gh_ps[:, 0:256], op=ALU.add)
                tz = st.tile([B, 256], F32, tag="tz")
                nc.scalar.activation(tz[:], rzsum[:], AF.Tanh, scale=0.5)
                # r*hn*2 = (tz_r + 1) * hn   [r = 0.5 + 0.5 tz_r]
                rhn2 = st.tile([B, HC], F32, tag="rhn2")
                nc.vector.scalar_tensor_tensor(
                    rhn2[:], tz[:, 0:128], 1.0, gh_ps[:, 256:384],
                    op0=ALU.add, op1=ALU.mult)
                npre = st.tile([B, HC], F32, tag="npre")
                nc.vector.scalar_tensor_tensor(
                    npre[:], rhn2[:], 0.5, gihe[:, 256:384], op0=ALU.mult, op1=ALU.add)
                n_t = st.tile([B, HC], F32, tag="nt")
                nc.scalar.activation(n_t[:], npre[:], AF.Tanh)
                # h_new = n + z*(h-n), z = 0.5 + 0.5 tz_z:
                hmn = st.tile([B, HC], F32, tag="hmn")
                nc.vector.tensor_tensor(hmn[:], hnat[:], n_t[:], op=ALU.subtract)
                zm2 = st.tile([B, HC], F32, tag="zm2")
                nc.vector.scalar_tensor_tensor(
                    zm2[:], tz[:, 128:256], 1.0, hmn[:], op0=ALU.add, op1=ALU.mult)
                nc.vector.scalar_tensor_tensor(
                    hnat[:], zm2[:], 0.5, n_t[:], op0=ALU.mult, op1=ALU.add)

                # h exchange: transpose own slice, AllGather, rebuild hT
                tr2_ps = ps.tile([HC, B], F32, tag="p64")
                nc.tensor.matmul(tr2_ps[:], hnat[:], ident_t[0:64, 0:64], is_transpose=True)
                hT_my = st.tile([HC, B], BF16, tag="hTmy")
                nc.vector.tensor_copy(hT_my[:], tr2_ps[:])
                h_in = dram.tile([HC * B], BF16, tag="hin")
                nc.sync.dma_start(h_in[:].rearrange("(p f) -> p f", p=HC), hT_my[:])
                h_out = dram.tile([NC * HC * B], BF16, tag="hout2")
                nc.gpsimd.collective_compute(
                    "AllGather", ALU.bypass, replica_groups=RG,
                    ins=[h_in.opt()], outs=[h_out.opt()])
                for cc in range(NC):
                    nc.sync.dma_start(
                        hT[:, cc, :],
                        h_out[cc * HC * B:(cc + 1) * HC * B].rearrange("(p f) -> p f", p=HC))
                nc.vector.tensor_copy(H_hist[:, :, t, :], hT[:])
                if t % 2 == 1:
                    m = t // 2
                    NW = VC // 8  # 500
                    for nbh in range(2):
                        lp_ps = ps.tile([128, NW], F32, tag="lpp", bufs=2)
                        for kc in range(KC):
                            nc.tensor.matmul(
                                lp_ps[:], H_hist[:, kc, 2 * m:2 * m + 2, :],
                                WoutA[:, kc, nbh * NW:(nbh + 1) * NW],
                                start=(kc == 0), stop=False)
                        nc.tensor.matmul(lp_ps[:], ones128[:],
                                         bout_t[:, nbh * NW:(nbh + 1) * NW],
                                         start=False, stop=True)
                        lsb = st.tile([128, NW], BF16, tag="lsb")
                        nc.scalar.copy(lsb[:], lp_ps[:])
                        nc.sync.dma_start(logits_dram[:, m, nbh * NW:(nbh + 1) * NW], lsb[:])
                        tmpm = st.tile([128, 1], F32, tag="tmpm")
                        nc.vector.tensor_reduce(tmpm[:], lsb[:],
                                                axis=mybir.AxisListType.X, op=ALU.max)
                        if nbh == 0:
                            nc.vector.tensor_copy(rmax[:, m:m + 1], tmpm[:])
                        else:
                            nc.vector.tensor_tensor(rmax[:, m:m + 1], rmax[:, m:m + 1],
                                                    tmpm[:], op=ALU.max)

            nc.sync.dma_start(hout[:], hnat[:])
            _ps_cm.__exit__(None, None, None)
            _st_cm.__exit__(None, None, None)
            # -------- phase C: logits + log-softmax --------
            with (
                tc.tile_pool(name="pc", bufs=2) as pc,
                tc.tile_pool(name="pc1", bufs=1) as pc1,
                tc.tile_pool(name="psC", bufs=2, space="PSUM") as psC,
            ):
                sume_c = pc1.tile([128, 16], F32, tag="sumec")
                NB = 8
                NW = VC // NB  # 500
                for nb in range(2, NB):
                    wtile = pc.tile([128, KC, NW], BF16, tag="wt2")
                    nc.sync.dma_start(
                        wtile[:],
                        WoutT[:].rearrange("(kc p) v -> p kc v", p=128)[:, :, nb * NW:(nb + 1) * NW])
                    for m in range(16):
                        lp_ps = psC.tile([128, NW], F32, tag="lpp")
                        for kc in range(KC):
                            nc.tensor.matmul(
                                lp_ps[:], H_hist[:, kc, 2 * m:2 * m + 2, :],
                                wtile[:, kc, :], start=(kc == 0), stop=False)
                        nc.tensor.matmul(lp_ps[:], ones128[:],
                                         bout_t[:, nb * NW:(nb + 1) * NW],
                                         start=False, stop=True)
                        lsb = pc.tile([128, NW], BF16, tag="lsb")
                        nc.scalar.copy(lsb[:], lp_ps[:])
                        nc.sync.dma_start(logits_dram[:, m, nb * NW:(nb + 1) * NW], lsb[:])
                        tmpm = pc.tile([128, 1], F32, tag="tmpm")
                        nc.vector.tensor_reduce(tmpm[:], lsb[:],
                                                axis=mybir.AxisListType.X, op=ALU.max)
                        nc.vector.tensor_tensor(rmax[:, m:m + 1], rmax[:, m:m + 1],
                                                tmpm[:], op=ALU.max)
                mx_in = dram.tile([128 * 16], F32, tag="mxin")
                nc.sync.dma_start(mx_in[:].rearrange("(p f) -> p f", p=128), rmax[:])
                mx_out = dram.tile([128 * 16], F32, tag="mxout")
                nc.gpsimd.collective_compute(
                    "AllReduce", ALU.max, replica_groups=RG,
                    ins=[mx_in.opt()], outs=[mx_out.opt()])
                gmax = pc1.tile([128, 16], F32, tag="gmax")
                nc.sync.dma_start(gmax[:], mx_out[:].rearrange("(p f) -> p f", p=128))
                gneg = pc1.tile([128, 16], F32, tag="gneg")
                nc.vector.tensor_scalar(gneg[:], gmax[:], -1.0, None, op0=ALU.mult)

                for m in range(16):
                    lt = pc.tile([128, VC], BF16, tag="lt")
                    nc.sync.dma_start(lt[:], logits_dram[:, m, :])
                    esc = pc.tile([128, VC], BF16, tag="wt2")
                    nc.scalar.activation(esc[:], lt[:], AF.Exp, bias=gneg[:, m:m + 1],
                                         accum_out=sume_c[:, m:m + 1])
                se_in = dram.tile([128 * 16], F32, tag="sein")
                nc.sync.dma_start(se_in[:].rearrange("(p f) -> p f", p=128), sume_c[:])
                se_out = dram.tile([128 * 16], F32, tag="seout")
                nc.gpsimd.collective_compute(
                    "AllReduce", ALU.add, replica_groups=RG,
                    ins=[se_in.opt()], outs=[se_out.opt()])
                gsum = pc1.tile([128, 16], F32, tag="gsum")
                nc.sync.dma_start(gsum[:], se_out[:].rearrange("(p f) -> p f", p=128))
                lnz = pc1.tile([128, 16], F32, tag="lnz")
                nc.scalar.activation(lnz[:], gsum[:], AF.Ln)
                nneg = pc1.tile([128, 16], F32, tag="nneg")
                nc.vector.tensor_tensor(nneg[:], gmax[:], lnz[:], op=ALU.add)
                nc.vector.tensor_scalar(nneg[:], nneg[:], -1.0, None, op0=ALU.mult)

                for m in range(16):
                    lt = pc.tile([128, VC], BF16, tag="lt")
                    nc.sync.dma_start(lt[:], logits_dram[:, m, :])
                    for h2 in range(2):
                        of = pc.tile([128, VC // 2], F32, tag="of")
                        nc.vector.tensor_scalar(
                            of[:], lt[:, h2 * (VC // 2):(h2 + 1) * (VC // 2)],
                            nneg[:, m:m + 1], None, op0=ALU.add)
                        for tt in range(2):
                            nc.sync.dma_start(
                                lp_part[:, 2 * m + tt,
                                        h2 * (VC // 2):(h2 + 1) * (VC // 2)],
                                of[64 * tt:64 * (tt + 1), :])
            _perB_cm.__exit__(None, None, None)

    nc.compile()
    _prog_cache["nc"] = nc
    return nc


def _host_prep(inputs):
    f = lambda x: np.asarray(x)
    enc = f(inputs["encoder_outputs"]).astype(np.float32)          # (B,S,H)
    h0 = f(inputs["encoder_hidden"]).astype(np.float32)[0]         # (B,H)
    tgt = f(inputs["target_tensor"])
    embedding = f(inputs["embedding"]).astype(np.float32)
    Wa, ba = f(inputs["Wa"]).astype(np.float32), f(inputs["ba"]).astype(np.float32)
    Ua, bu = f(inputs["Ua"]).astype(np.float32), f(inputs["bu"]).astype(np.float32)
    Va = f(inputs["Va"]).astype(np.float32)
    W_ih, b_ih = f(inputs["W_ih"]).astype(np.float32), f(inputs["b_ih"]).astype(np.float32)
    W_hh, b_hh = f(inputs["W_hh"]).astype(np.float32), f(inputs["b_hh"]).astype(np.float32)
    W_out, b_out = f(inputs["W_out"]).astype(np.float32), f(inputs["b_out"]).astype(np.float32)

    tokens = np.concatenate([np.full((B, 1), SOS, tgt.dtype), tgt[:, :-1]], axis=1)
    emb = embedding[tokens]                                        # (B,T,H)

    encT = np.ascontiguousarray(enc.transpose(2, 0, 1).reshape(H, BS)).astype(bf16)
    embT = np.ascontiguousarray(emb.transpose(2, 1, 0).reshape(H, TB)).astype(bf16)
    h0T = np.ascontiguousarray(
        h0.T.reshape(KC, 128, B).transpose(1, 0, 2)).astype(bf16)  # (128,kc,b)

    in_maps = []
    for c in range(NC):
        gs = slice(c * HC, (c + 1) * HC)
        rows = np.r_[c * HC:(c + 1) * HC, H + c * HC:H + (c + 1) * HC,
                     2 * H + c * HC:2 * H + (c + 1) * HC]
        vs = slice(c * VC, (c + 1) * VC)
        in_maps.append({
            "encT": encT,
            "UaT": np.ascontiguousarray(Ua[gs, :].T).astype(bf16),
            "WaT": np.ascontiguousarray(Wa[gs, :].T).astype(bf16),
            "kbias": (ba + bu)[gs].reshape(HC, 1).astype(np.float32),
            "Vaw": Va[0, gs].reshape(HC, 1).astype(bf16),
            "embT": embT,
            "Wih1T": np.ascontiguousarray(W_ih[rows, :H].T).astype(bf16),
            "Wih2T": np.ascontiguousarray(W_ih[rows, H:].T).astype(bf16),
            "WhhT": np.ascontiguousarray(W_hh[rows, :].T).astype(bf16),
            "gbias": (b_ih[rows] + np.concatenate(
                [b_hh[c * HC:(c + 1) * HC], b_hh[H + c * HC:H + (c + 1) * HC],
                 np.zeros(HC, np.float32)])).reshape(1, GC).astype(bf16),
            "hbias": np.concatenate(
                [np.zeros(2 * HC, np.float32),
                 b_hh[2 * H + c * HC:2 * H + (c + 1) * HC]]).reshape(1, GC).astype(bf16),
            "enc_hs": np.ascontiguousarray(enc[:, :, gs].transpose(1, 0, 2)).astype(bf16),
            "h0T": h0T,
            "h0nat": np.ascontiguousarray(h0[:, gs]).astype(np.float32),
            "WoutT": np.ascontiguousarray(W_out[vs, :].T).astype(bf16),
            "bout": b_out[vs].reshape(1, VC).astype(bf16),
            "ident": np.eye(128, dtype=np.float32),
        })
    return in_maps


def _run(inputs, trace=False):
    nc = _build_program()
    in_maps = _host_prep(inputs)
    res = run_bass_kernel_spmd(nc, in_maps, list(range(NC)), trace=trace)
    r = res.results
    lp = np.concatenate([r[c]["lp_part"] for c in range(NC)], axis=2)
    h_final = np.concatenate([r[c]["hout"] for c in range(NC)], axis=1)[None]
    attentions = r[0]["attn"]
    return (lp, h_final, attentions), res


def kernel(**inputs):
    out, _ = _run(inputs, trace=False)
    return out


def bench(inputs, iters=3):
    """Time device execution with pre-staged inputs (excludes host transfers)."""
    import time as _time
    import jax
    from jax.sharding import Mesh, PartitionSpec
    from jax.experimental.shard_map import shard_map
    from concourse import bass2jax as b2j
    from concourse.bass2jax import _bass_exec_p, partition_id_tensor
    import concourse.mybir as _mybir

    nc = _build_program()
    in_maps = _host_prep(inputs)
    n_cores = NC
    b2j.install_neuronx_cc_hook()
    partition_name = nc.partition_id_tensor.name if nc.partition_id_tensor else None
    in_names, out_names, out_avals, zero_outs = [], [], [], []
    for alloc in nc.m.functions[0].allocations:
        if not isinstance(alloc, _mybir.MemoryLocationSet):
            continue
        name = alloc.memorylocations[0].name
        if alloc.kind == "ExternalInput":
            if name != partition_name:
                in_names.append(name)
        elif alloc.kind == "ExternalOutput":
            import numpy as _np
            dt = _mybir.dt.np(alloc.dtype)
            out_names.append(name)
            out_avals.append(jax.core.ShapedArray(tuple(alloc.tensor_shape), dt))
            zero_outs.append(_np.zeros(tuple(alloc.tensor_shape), dt))
    n_params = len(in_names)
    n_outs = len(out_avals)
    in_names = in_names + out_names + ([partition_name] if partition_name else [])
    donate = tuple(range(n_params, n_params + n_outs))

    def _body(*args):
        operands = list(args)
        if partition_name is not None:
            operands.append(partition_id_tensor())
        return tuple(_bass_exec_p.bind(
            *operands, out_avals=tuple(out_avals), in_names=tuple(in_names),
            out_names=tuple(out_names), lowering_input_output_aliases=(),
            sim_require_finite=True, sim_require_nnan=True, nc=nc))

    devices = jax.devices()[:n_cores]
    mesh = Mesh(np.asarray(devices), ("core",))
    sharded = jax.jit(
        shard_map(_body, mesh=mesh,
                  in_specs=(PartitionSpec("core"),) * (n_params + n_outs),
                  out_specs=(PartitionSpec("core"),) * n_outs,
                  check_rep=False),
        donate_argnums=donate, keep_unused=True)
    per_core = [[np.asarray(m[nm]) for nm in in_names[:n_params]] for m in in_maps]
    concat_in = [np.concatenate([per_core[c][i] for c in range(n_cores)], axis=0)
                 for i in range(n_params)]
    from jax.sharding import NamedSharding
    shard = NamedSharding(mesh, PartitionSpec("core"))
    dev_in = [jax.device_put(x, shard) for x in concat_in]
    times = []
    out = None
    for it in range(iters + 1):
        zs = [jax.device_put(np.zeros((n_cores * z.shape[0], *z.shape[1:]), z.dtype), shard)
              for z in zero_outs]
        jax.block_until_ready(zs)
        t0 = _time.perf_counter()
        out = sharded(*dev_in, *zs)
        jax.block_until_ready(out)
        t1 = _time.perf_counter()
        times.append(t1 - t0)
    return times, out_names, out


# revision 15
# speedup vs baseline: 1.9316x; 1.0646x over previous
"""AttnDecoderRNN Trainium2 kernel: 8-core SPMD.

Sharding:
  - Attention g-dim, ctx h-dim, GRU gate n-dim: column-sharded 128/core.
  - Per step: AllReduce(partial scores), AllGather(ctx^T slice), AllGather(h^T slice).
  - Output projection: vocab-sharded 4000/core; batched (B*T, H) @ W_out^T after the
    recurrence with a distributed log-softmax (AllReduce max, AllReduce sumexp).
Host does embedding gather, transposes/slices, and final concatenation only.
"""
import sys
import numpy as np

sys.path.insert(0, "/opt/trn_rl_repo")

import ml_dtypes
from concourse import bass, bacc, mybir
from concourse import tile
from concourse.bass_utils import run_bass_kernel_spmd

bf16 = ml_dtypes.bfloat16
F32 = mybir.dt.float32
BF16 = mybir.dt.bfloat16
AF = mybir.ActivationFunctionType
ALU = mybir.AluOpType

B, S, T, H, V = 64, 128, 32, 1024, 32000
NC = 8
HC = H // NC          # 128  per-core hidden/g slice
GC = 3 * H // NC      # 384  per-core gate rows
VC = V // NC          # 4000 per-core vocab slice
KC = H // 128         # 8    K-chunks
BS = B * S            # 8192
TB = T * B            # 2048
SOS = 1
RG = [list(range(NC))]

_prog_cache = {}


def _build_program():
    if "nc" in _prog_cache:
        return _prog_cache["nc"]
    nc = bacc.Bacc("TRN2", target_bir_lowering=False, debug=False, num_devices=8)

    # ---------------- inputs (per-core content) ----------------
    encT = nc.declare_dram_parameter("encT", [H, BS], BF16, isOutput=False)
    UaT = nc.declare_dram_parameter("UaT", [H, HC], BF16, isOutput=False)
    WaT = nc.declare_dram_parameter("WaT", [H, HC], BF16, isOutput=False)
    kbias = nc.declare_dram_parameter("kbias", [HC, 1], F32, isOutput=False)
    Vaw = nc.declare_dram_parameter("Vaw", [HC, 1], BF16, isOutput=False)
    embT = nc.declare_dram_parameter("embT", [H, TB], BF16, isOutput=False)
    Wih1T = nc.declare_dram_parameter("Wih1T", [H, GC], BF16, isOutput=False)
    Wih2T = nc.declare_dram_parameter("Wih2T", [H, GC], BF16, isOutput=False)
    WhhT = nc.declare_dram_parameter("WhhT", [H, GC], BF16, isOutput=False)
    gbias = nc.declare_dram_parameter("gbias", [1, GC], BF16, isOutput=False)
    hbias = nc.declare_dram_parameter("hbias", [1, GC], BF16, isOutput=False)
    enc_hs = nc.declare_dram_parameter("enc_hs", [S, B, HC], BF16, isOutput=False)
    h0T = nc.declare_dram_parameter("h0T", [128, KC, B], BF16, isOutput=False)
    h0nat = nc.declare_dram_parameter("h0nat", [B, HC], F32, isOutput=False)
    WoutT = nc.declare_dram_parameter("WoutT", [H, VC], BF16, isOutput=False)
    bout = nc.declare_dram_parameter("bout", [1, VC], BF16, isOutput=False)
    ident = nc.declare_dram_parameter("ident", [128, 128], F32, isOutput=False)

    lp_part = nc.declare_dram_parameter("lp_part", [B, T, VC], F32, isOutput=True)
    attn = nc.declare_dram_parameter("attn", [B, T, S], F32, isOutput=True)
    hout = nc.declare_dram_parameter("hout", [B, HC], F32, isOutput=True)

    with tile.TileContext(nc) as tc:
        with (
            tc.tile_pool(name="per", bufs=1) as per,
            tc.tile_pool(name="dram", bufs=2, space="DRAM") as dram,
        ):
            # -------- persistent tiles --------
            keys_t = per.tile([128, B, S], BF16, tag="keys")       # (g, b, s)
            gi_emb = per.tile([B, T, GC], F32, tag="giemb")        # emb@Wih1T per t
            enc_hs_t = per.tile([S, B, HC], BF16, tag="enchs")
            WaT_t = per.tile([128, KC, HC], BF16, tag="WaT")
            WhhT_t = per.tile([128, KC, GC], BF16, tag="WhhT")
            Wih2T_t = per.tile([128, KC, GC], BF16, tag="Wih2T")
            Va_t = per.tile([HC, 1], BF16, tag="Va")
            kbias_t = per.tile([HC, 1], F32, tag="kbias")
            gbias_t = per.tile([1, GC], BF16, tag="gbias")
            hbias_t = per.tile([1, GC], BF16, tag="hbias")
            ident_t = per.tile([128, 128], F32, tag="ident")
            ones64 = per.tile([1, B], BF16, tag="ones64")
            ones128 = per.tile([1, 128], BF16, tag="ones128")
            bout_t = per.tile([1, VC], BF16, tag="bout")
            hT = per.tile([128, KC, B], BF16, tag="hT")            # full h^T
            hnat = per.tile([B, HC], F32, tag="hnat")              # own h slice
            ctxTf = per.tile([128, KC, B], BF16, tag="ctxTf")      # full ctx^T
            H_hist = per.tile([128, KC, T, B], BF16, tag="Hh")     # h^T history

            nc.vector.memset(ones64[:], 1.0)
            nc.vector.memset(ones128[:], 1.0)
            nc.sync.dma_start(WaT_t[:], WaT[:].rearrange("(kc p) n -> p kc n", p=128))
            nc.sync.dma_start(WhhT_t[:], WhhT[:].rearrange("(kc p) n -> p kc n", p=128))
            nc.sync.dma_start(Wih2T_t[:], Wih2T[:].rearrange("(kc p) n -> p kc n", p=128))
            nc.sync.dma_start(Va_t[:], Vaw[:])
            nc.sync.dma_start(kbias_t[:], kbias[:])
            nc.sync.dma_start(gbias_t[:], gbias[:])
            nc.sync.dma_start(hbias_t[:], hbias[:])
            nc.sync.dma_start(ident_t[:], ident[:])
            nc.sync.dma_start(bout_t[:], bout[:])
            nc.sync.dma_start(enc_hs_t[:], enc_hs[:])
            nc.sync.dma_start(hT[:], h0T[:])
            nc.sync.dma_start(hnat[:], h0nat[:])
            logits_dram = dram.tile([128, 16, VC], BF16, tag="logits")

            # -------- phase A1: gi_emb = emb @ Wih1T (per t) --------
            with (
                tc.tile_pool(name="bootA", bufs=1) as bootA,
                tc.tile_pool(name="psA", bufs=2, space="PSUM") as psA,
            ):
                embT_t = bootA.tile([128, KC, TB], BF16, tag="embT")
                Wih1T_t = bootA.tile([128, KC, GC], BF16, tag="Wih1T")
                nc.sync.dma_start(embT_t[:], embT[:].rearrange("(kc p) n -> p kc n", p=128))
                nc.sync.dma_start(Wih1T_t[:], Wih1T[:].rearrange("(kc p) n -> p kc n", p=128))
                for t_ in range(T):
                    gp = psA.tile([B, GC], F32, tag="gp")
                    for kc in range(KC):
                        nc.tensor.matmul(
                            gp[:], embT_t[:, kc, t_ * B:(t_ + 1) * B],
                            Wih1T_t[:, kc, :], start=(kc == 0), stop=(kc == KC - 1))
                    nc.vector.tensor_copy(gi_emb[:, t_, :], gp[:])

            # -------- phase A2: keys = (enc @ Ua^T)^T + (ba+bu) --------
            with (
                tc.tile_pool(name="bootB", bufs=2) as bootB,
                tc.tile_pool(name="psB", bufs=2, space="PSUM") as psB,
            ):
                UaT_t = bootB.tile([128, KC, HC], BF16, tag="UaT")
                nc.sync.dma_start(UaT_t[:], UaT[:].rearrange("(kc p) n -> p kc n", p=128))
                keys_flat = keys_t[:].rearrange("p b s -> p (b s)")
                for blk in range(4):
                    encb = bootB.tile([128, KC, 2048], BF16, tag="encb")
                    nc.sync.dma_start(
                        encb[:],
                        encT[:].rearrange("(kc p) n -> p kc n", p=128)[:, :, blk * 2048:(blk + 1) * 2048])
                    for nbl in range(4):
                        kp = psB.tile([128, 512], F32, tag="kp")
                        for kc in range(KC):
                            nc.tensor.matmul(
                                kp[:], UaT_t[:, kc, :],
                                encb[:, kc, nbl * 512:(nbl + 1) * 512],
                                start=(kc == 0), stop=(kc == KC - 1))
                        nc.vector.tensor_scalar(
                            keys_flat[:, blk * 2048 + nbl * 512: blk * 2048 + (nbl + 1) * 512],
                            kp[:], kbias_t[:], None, op0=ALU.add)

            # -------- phase B: recurrence --------
            _perB_cm = tc.tile_pool(name="perB", bufs=1)
            perB = _perB_cm.__enter__()
            WoutA = perB.tile([128, KC, VC // 4], BF16, tag="WoutA")
            rmax = perB.tile([128, 16], F32, tag="rmax")
            nc.sync.dma_start(
                WoutA[:],
                WoutT[:].rearrange("(kc p) v -> p kc v", p=128)[:, :, 0:VC // 4])
            _st_cm = tc.tile_pool(name="st", bufs=2)
            _ps_cm = tc.tile_pool(name="psmain", bufs=2, space="PSUM")
            st = _st_cm.__enter__()
            ps = _ps_cm.__enter__()
            for t in range(T):
                # q^T = (Wa_c h)^T  -> (128g, 64b)
                q_ps = ps.tile([128, B], F32, tag="p64")
                for kc in range(KC):
                    nc.tensor.matmul(q_ps[:], WaT_t[:, kc, :], hT[:, kc, :],
                                     start=(kc == 0), stop=(kc == KC - 1))
                qT_bf = st.tile([128, B], BF16, tag="qT")
                nc.vector.tensor_copy(qT_bf[:], q_ps[:])

                # add + tanh + Va matvecs, chunked over b (16 b per chunk)
                sc_in = dram.tile([4 * 2048], F32, tag="scin")
                for rnd in range(2):
                    sc_ps = ps.tile([128, 1024], F32, tag="scp", bufs=1)
                    for ch in range(2):
                        bsl = slice(rnd * 32 + ch * 16, rnd * 32 + (ch + 1) * 16)
                        add_t = st.tile([128, 16, S], BF16, tag="addt")
                        nc.vector.tensor_tensor(
                            add_t[:], keys_t[:, bsl, :],
                            qT_bf[:, bsl].unsqueeze(2).broadcast_to([128, 16, S]),
                            op=ALU.add)
                        tanh_t = st.tile([128, 16, S], BF16, tag="tanht")
                        nc.scalar.activation(tanh_t[:], add_t[:], AF.Tanh)
                        for bl in range(16):
                            b = rnd * 32 + ch * 16 + bl
                            j, k = b % 4, (b // 4) % 8
                            nc.tensor.matmul(
                                sc_ps[32 * j:32 * j + 1, k * S:(k + 1) * S],
                                Va_t[:], tanh_t[:, bl, :], start=True, stop=True,
                                tile_position=(0, 32 * j))
                    scp_s = st.tile([128, 1024], F32, tag="scps")
                    nc.vector.tensor_copy(scp_s[:], sc_ps[:])
                    # b-major bounce: idx = (k*4+j)*S + s within this round's half
                    nc.sync.dma_start(
                        sc_in[rnd * 4096:(rnd + 1) * 4096].rearrange(
                            "(k j s) -> j k s", j=4, s=S),
                        scp_s[:].rearrange("(j r) (k s) -> j r k s", r=32, s=S)[:, 0, :, :])
                sc_out = dram.tile([NC * 4 * 2048], F32, tag="scout")
                nc.gpsimd.collective_compute(
                    "AllGather", ALU.bypass, replica_groups=RG,
                    ins=[sc_in.opt()], outs=[sc_out.opt()])
                scg = st.tile([B, NC, S], F32, tag="scg")
                nc.sync.dma_start(
                    scg[:], sc_out[:].rearrange("(c b s) -> b c s", s=S, c=NC))
                sc_t = st.tile([B, S], F32, tag="sct")
                nc.vector.tensor_tensor(scg[:, 0, :], scg[:, 0, :], scg[:, 4, :], op=ALU.add)
                nc.vector.tensor_tensor(scg[:, 1, :], scg[:, 1, :], scg[:, 5, :], op=ALU.add)
                nc.vector.tensor_tensor(scg[:, 2, :], scg[:, 2, :], scg[:, 6, :], op=ALU.add)
                nc.vector.tensor_tensor(scg[:, 3, :], scg[:, 3, :], scg[:, 7, :], op=ALU.add)
                nc.vector.tensor_tensor(scg[:, 0, :], scg[:, 0, :], scg[:, 2, :], op=ALU.add)
                nc.vector.tensor_tensor(scg[:, 1, :], scg[:, 1, :], scg[:, 3, :], op=ALU.add)
                nc.vector.tensor_tensor(sc_t[:], scg[:, 0, :], scg[:, 1, :], op=ALU.add)

                # softmax -> w (64, 128)
                negmax = st.tile([B, 1], F32, tag="negmax")
                nc.vector.tensor_reduce(negmax[:], sc_t[:], axis=mybir.AxisListType.X,
                                        op=ALU.max, negate=True)
                w_t = st.tile([B, S], F32, tag="wt")
                sume = st.tile([B, 1], F32, tag="sume")
                nc.scalar.activation(w_t[:], sc_t[:], AF.Exp, bias=negmax[:],
                                     accum_out=sume[:])
                recip = st.tile([B, 1], F32, tag="recip")
                nc.vector.reciprocal(recip[:], sume[:])
                nc.vector.tensor_scalar(w_t[:], w_t[:], recip[:], None, op0=ALU.mult)
                nc.sync.dma_start(attn[:, t, :], w_t[:])

                # w^T (128s, 64b) bf16 via PE transpose
                tr_ps = ps.tile([S, B], F32, tag="p64")
                nc.tensor.matmul(tr_ps[:], w_t[:], ident_t[0:64, 0:64], is_transpose=True)
                wT_bf = st.tile([S, B], BF16, tag="wTb")
                nc.vector.tensor_copy(wT_bf[:], tr_ps[:])

                # ctx^T slice (128h, 64b): per-b matvec enc_hs[b]^T w[b]
                ctx_ps = ps.tile([HC, B], F32, tag="p64")
                for b in range(B):
                    nc.tensor.matmul(ctx_ps[:, b:b + 1], enc_hs_t[:, b, :],
                                     wT_bf[:, b:b + 1], start=True, stop=True)
                ctxT_bf = st.tile([HC, B], BF16, tag="ctxTb")
                nc.vector.tensor_copy(ctxT_bf[:], ctx_ps[:])
                ctx_in = dram.tile([HC * B], BF16, tag="ctxin")
                nc.sync.dma_start(ctx_in[:].rearrange("(p f) -> p f", p=HC), ctxT_bf[:])
                ctx_out = dram.tile([NC * HC * B], BF16, tag="ctxout")
                nc.gpsimd.collective_compute(
                    "AllGather", ALU.bypass, replica_groups=RG,
                    ins=[ctx_in.opt()], outs=[ctx_out.opt()])
                nc.sync.dma_start(
                    ctxTf[:],
                    ctx_out[:].rearrange("(c p f) -> p c f", p=HC, f=B))

                # gates
                gi_ps = ps.tile([B, GC], F32, tag="p384")
                for kc in range(KC):
                    nc.tensor.matmul(gi_ps[:], ctxTf[:, kc, :], Wih2T_t[:, kc, :],
                                     start=(kc == 0), stop=False)
                nc.tensor.matmul(gi_ps[:], ones64[:], gbias_t[:], start=False, stop=True)
                gh_ps = ps.tile([B, GC], F32, tag="p384")
                for kc in range(KC):
                    nc.tensor.matmul(gh_ps[:], hT[:, kc, :], WhhT_t[:, kc, :],
                                     start=(kc == 0), stop=False)
                nc.tensor.matmul(gh_ps[:], ones64[:], hbias_t[:], start=False, stop=True)
                gihe = st.tile([B, GC], F32, tag="gihe")
                nc.vector.scalar_tensor_tensor(
                    gihe[:], gi_ps[:], 1.0, gi_emb[:, t, :], op0=ALU.mult, op1=ALU.add)
                rzsum = st.tile([B, 256], F32, tag="rzsum")
                nc.vector.tensor_tensor(rzsum[:], gihe[:, 0:256], gh_ps[:, 0:256], op=ALU.add)
                tz = st.tile([B, 256], F32, tag="tz")
                nc.scalar.activation(tz[:], rzsum[:], AF.Tanh, scale=0.5)
                # r*hn*2 = (tz_r + 1) * hn   [r = 0.5 + 0.5 tz_r]
                rhn2 = st.tile([B, HC], F32, tag="rhn2")
                nc.vector.scalar_tensor_tensor(
                    rhn2[:], tz[:, 0:128], 1.0, gh_ps[:, 256:384],
                    op0=ALU.add, op1=ALU.mult)
                npre = st.tile([B, HC], F32, tag="npre")
                nc.vector.scalar_tensor_tensor(
                    npre[:], rhn2[:], 0.5, gihe[:, 256:384], op0=ALU.mult, op1=ALU.add)
                n_t = st.tile([B, HC], F32, tag="nt")
                nc.scalar.activation(n_t[:], npre[:], AF.Tanh)
                # h_new = n + z*(h-n), z = 0.5 + 0.5 tz_z:
                hmn = st.tile([B, HC], F32, tag="hmn")
                nc.vector.tensor_tensor(hmn[:], hnat[:], n_t[:], op=ALU.subtract)
                zm2 = st.tile([B, HC], F32, tag="zm2")
                nc.vector.scalar_tensor_tensor(
                    zm2[:], tz[:, 128:256], 1.0, hmn[:], op0=ALU.add, op1=ALU.mult)
                nc.vector.scalar_tensor_tensor(
                    hnat[:], zm2[:], 0.5, n_t[:], op0=ALU.mult, op1=ALU.add)

                # h exchange: transpose own slice, AllGather, rebuild hT
                tr2_ps = ps.tile([HC, B], F32, tag="p64")
                nc.tensor.matmul(tr2_ps[:], hnat[:], ident_t[0:64, 0:64], is_transpose=True)
                hT_my = st.tile([HC, B], BF16, tag="hTmy")
                nc.vector.tensor_copy(hT_my[:], tr2_ps[:])
                h_in = dram.tile([HC * B], BF16, tag="hin")
                nc.sync.dma_start(h_in[:].rearrange("(p f) -> p f", p=HC), hT_my[:])
                h_out = dram.tile([NC * HC * B], BF16, tag="hout2")
                nc.gpsimd.collective_compute(
                    "AllGather", ALU.bypass, replica_groups=RG,
                    ins=[h_in.opt()], outs=[h_out.opt()])
                nc.sync.dma_start(
                    hT[:],
                    h_out[:].rearrange("(c p f) -> p c f", p=HC, f=B))
                nc.vector.tensor_copy(H_hist[:, :, t, :], hT[:])
                if t % 2 == 1:
                    m = t // 2
                    NW = VC // 8  # 500
                    for nbh in range(2):
                        lp_ps = ps.tile([128, NW], F32, tag="lpp", bufs=2)
                        for kc in range(KC):
                            nc.tensor.matmul(
                                lp_ps[:], H_hist[:, kc, 2 * m:2 * m + 2, :],
                                WoutA[:, kc, nbh * NW:(nbh + 1) * NW],
                                start=(kc == 0), stop=False)
                        nc.tensor.matmul(lp_ps[:], ones128[:],
                                         bout_t[:, nbh * NW:(nbh + 1) * NW],
                                         start=False, stop=True)
                        lsb = st.tile([128, NW], BF16, tag="lsb")
                        nc.scalar.copy(lsb[:], lp_ps[:])
                        nc.sync.dma_start(logits_dram[:, m, nbh * NW:(nbh + 1) * NW], lsb[:])
                        tmpm = st.tile([128, 1], F32, tag="tmpm")
                        nc.vector.tensor_reduce(tmpm[:], lsb[:],
                                                axis=mybir.AxisListType.X, op=ALU.max)
                        if nbh == 0:
                            nc.vector.tensor_copy(rmax[:, m:m + 1], tmpm[:])
                        else:
                            nc.vector.tensor_tensor(rmax[:, m:m + 1], rmax[:, m:m + 1],
                                                    tmpm[:], op=ALU.max)

            nc.sync.dma_start(hout[:], hnat[:])
            _ps_cm.__exit__(None, None, None)
            _st_cm.__exit__(None, None, None)
            # -------- phase C: logits + log-softmax --------
            with (
                tc.tile_pool(name="pc", bufs=2) as pc,
                tc.tile_pool(name="pc1", bufs=1) as pc1,
                tc.tile_pool(name="psC", bufs=2, space="PSUM") as psC,
            ):
                sume_c = pc1.tile([128, 16], F32, tag="sumec")
                NB = 8
                NW = VC // NB  # 500
                for nb in range(2, NB):
                    wtile = pc.tile([128, KC, NW], BF16, tag="wt2")
                    nc.sync.dma_start(
                        wtile[:],
                        WoutT[:].rearrange("(kc p) v -> p kc v", p=128)[:, :, nb * NW:(nb + 1) * NW])
                    for m in range(16):
                        lp_ps = psC.tile([128, NW], F32, tag="lpp")
                        for kc in range(KC):
                            nc.tensor.matmul(
                                lp_ps[:], H_hist[:, kc, 2 * m:2 * m + 2, :],
                                wtile[:, kc, :], start=(kc == 0), stop=False)
                        nc.tensor.matmul(lp_ps[:], ones128[:],
                                         bout_t[:, nb * NW:(nb + 1) * NW],
                                         start=False, stop=True)
                        lsb = pc.tile([128, NW], BF16, tag="lsb")
                        nc.scalar.copy(lsb[:], lp_ps[:])
                        nc.sync.dma_start(logits_dram[:, m, nb * NW:(nb + 1) * NW], lsb[:])
                        tmpm = pc.tile([128, 1], F32, tag="tmpm")
                        nc.vector.tensor_reduce(tmpm[:], lsb[:],
                                                axis=mybir.AxisListType.X, op=ALU.max)
                        nc.vector.tensor_tensor(rmax[:, m:m + 1], rmax[:, m:m + 1],
                                                tmpm[:], op=ALU.max)
                mx_in = dram.tile([128 * 16], F32, tag="mxin")
                nc.sync.dma_start(mx_in[:].rearrange("(p f) -> p f", p=128), rmax[:])
                mx_out = dram.tile([128 * 16], F32, tag="mxout")
                nc.gpsimd.collective_compute(
                    "AllReduce", ALU.max, replica_groups=RG,
                    ins=[mx_in.opt()], outs=[mx_out.opt()])
                gmax = pc1.tile([128, 16], F32, tag="gmax")
                nc.sync.dma_start(gmax[:], mx_out[:].rearrange("(p f) -> p f", p=128))
                gneg = pc1.tile([128, 16], F32, tag="gneg")
                nc.vector.tensor_scalar(gneg[:], gmax[:], -1.0, None, op0=ALU.mult)

                for m in range(16):
                    lt = pc.tile([128, VC], BF16, tag="lt")
                    nc.sync.dma_start(lt[:], logits_dram[:, m, :])
                    esc = pc.tile([128, VC], BF16, tag="wt2")
                    nc.scalar.activation(esc[:], lt[:], AF.Exp, bias=gneg[:, m:m + 1],
                                         accum_out=sume_c[:, m:m + 1])
                se_in = dram.tile([128 * 16], F32, tag="sein")
                nc.sync.dma_start(se_in[:].rearrange("(p f) -> p f", p=128), sume_c[:])
                se_out = dram.tile([128 * 16], F32, tag="seout")
                nc.gpsimd.collective_compute(
                    "AllReduce", ALU.add, replica_groups=RG,
                    ins=[se_in.opt()], outs=[se_out.opt()])
                gsum = pc1.tile([128, 16], F32, tag="gsum")
                nc.sync.dma_start(gsum[:], se_out[:].rearrange("(p f) -> p f", p=128))
                lnz = pc1.tile([128, 16], F32, tag="lnz")
                nc.scalar.activation(lnz[:], gsum[:], AF.Ln)
                nneg = pc1.tile([128, 16], F32, tag="nneg")
                nc.vector.tensor_tensor(nneg[:], gmax[:], lnz[:], op=ALU.add)
                nc.vector.tensor_scalar(nneg[:], nneg[:], -1.0, None, op0=ALU.mult)

                for m in range(16):
                    lt = pc.tile([128, VC], BF16, tag="lt")
                    nc.sync.dma_start(lt[:], logits_dram[:, m, :])
                    for h2 in range(2):
                        of = pc.tile([128, VC // 2], F32, tag="of")
                        nc.vector.tensor_scalar(
                            of[:], lt[:, h2 * (VC // 2):(h2 + 1) * (VC // 2)],
                            nneg[:, m:m + 1], None, op0=ALU.add)
                        for tt in range(2):
                            nc.sync.dma_start(
                                lp_part[:, 2 * m + tt,
                                        h2 * (VC // 2):(h2 + 1) * (VC // 2)],
                                of[64 * tt:64 * (tt + 1), :])
            _perB_cm.__exit__(None, None, None)

    nc.compile()
    _prog_cache["nc"] = nc
    return nc


def _host_prep(inputs):
    f = lambda x: np.asarray(x)
    enc = f(inputs["encoder_outputs"]).astype(np.float32)          # (B,S,H)
    h0 = f(inputs["encoder_hidden"]).astype(np.float32)[0]         # (B,H)
    tgt = f(inputs["target_tensor"])
    embedding = f(inputs["embedding"]).astype(np.float32)
    Wa, ba = f(inputs["Wa"]).astype(np.float32), f(inputs["ba"]).astype(np.float32)
    Ua, bu = f(inputs["Ua"]).astype(np.float32), f(inputs["bu"]).astype(np.float32)
    Va = f(inputs["Va"]).astype(np.float32)
    W_ih, b_ih = f(inputs["W_ih"]).astype(np.float32), f(inputs["b_ih"]).astype(np.float32)
    W_hh, b_hh = f(inputs["W_hh"]).astype(np.float32), f(inputs["b_hh"]).astype(np.float32)
    W_out, b_out = f(inputs["W_out"]).astype(np.float32), f(inputs["b_out"]).astype(np.float32)

    tokens = np.concatenate([np.full((B, 1), SOS, tgt.dtype), tgt[:, :-1]], axis=1)
    emb = embedding[tokens]                                        # (B,T,H)

    encT = np.ascontiguousarray(enc.transpose(2, 0, 1).reshape(H, BS)).astype(bf16)
    embT = np.ascontiguousarray(emb.transpose(2, 1, 0).reshape(H, TB)).astype(bf16)
    h0T = np.ascontiguousarray(
        h0.T.reshape(KC, 128, B).transpose(1, 0, 2)).astype(bf16)  # (128,kc,b)

    in_maps = []
    for c in range(NC):
        gs = slice(c * HC, (c + 1) * HC)
        rows = np.r_[c * HC:(c + 1) * HC, H + c * HC:H + (c + 1) * HC,
                     2 * H + c * HC:2 * H + (c + 1) * HC]
        vs = slice(c * VC, (c + 1) * VC)
        in_maps.append({
            "encT": encT,
            "UaT": np.ascontiguousarray(Ua[gs, :].T).astype(bf16),
            "WaT": np.ascontiguousarray(Wa[gs, :].T).astype(bf16),
            "kbias": (ba + bu)[gs].reshape(HC, 1).astype(np.float32),
            "Vaw": Va[0, gs].reshape(HC, 1).astype(bf16),
            "embT": embT,
            "Wih1T": np.ascontiguousarray(W_ih[rows, :H].T).astype(bf16),
            "Wih2T": np.ascontiguousarray(W_ih[rows, H:].T).astype(bf16),
            "WhhT": np.ascontiguousarray(W_hh[rows, :].T).astype(bf16),
            "gbias": (b_ih[rows] + np.concatenate(
                [b_hh[c * HC:(c + 1) * HC], b_hh[H + c * HC:H + (c + 1) * HC],
                 np.zeros(HC, np.float32)])).reshape(1, GC).astype(bf16),
            "hbias": np.concatenate(
                [np.zeros(2 * HC, np.float32),
                 b_hh[2 * H + c * HC:2 * H + (c + 1) * HC]]).reshape(1, GC).astype(bf16),
            "enc_hs": np.ascontiguousarray(enc[:, :, gs].transpose(1, 0, 2)).astype(bf16),
            "h0T": h0T,
            "h0nat": np.ascontiguousarray(h0[:, gs]).astype(np.float32),
            "WoutT": np.ascontiguousarray(W_out[vs, :].T).astype(bf16),
            "bout": b_out[vs].reshape(1, VC).astype(bf16),
            "ident": np.eye(128, dtype=np.float32),
        })
    return in_maps


def _run(inputs, trace=False):
    nc = _build_program()
    in_maps = _host_prep(inputs)
    res = run_bass_kernel_spmd(nc, in_maps, list(range(NC)), trace=trace)
    r = res.results
    lp = np.concatenate([r[c]["lp_part"] for c in range(NC)], axis=2)
    h_final = np.concatenate([r[c]["hout"] for c in range(NC)], axis=1)[None]
    attentions = r[0]["attn"]
    return (lp, h_final, attentions), res


def kernel(**inputs):
    out, _ = _run(inputs, trace=False)
    return out


def bench(inputs, iters=3):
    """Time device execution with pre-staged inputs (excludes host transfers)."""
    import time as _time
    import jax
    from jax.sharding import Mesh, PartitionSpec
    from jax.experimental.shard_map import shard_map
    from concourse import bass2jax as b2j
    from concourse.bass2jax import _bass_exec_p, partition_id_tensor
    import concourse.mybir as _mybir

    nc = _build_program()
    in_maps = _host_prep(inputs)
    n_cores = NC
    b2j.install_neuronx_cc_hook()
    partition_name = nc.partition_id_tensor.name if nc.partition_id_tensor else None
    in_names, out_names, out_avals, zero_outs = [], [], [], []
    for alloc in nc.m.functions[0].allocations:
        if not isinstance(alloc, _mybir.MemoryLocationSet):
            continue
        name = alloc.memorylocations[0].name
        if alloc.kind == "ExternalInput":
            if name != partition_name:
                in_names.append(name)
        elif alloc.kind == "ExternalOutput":
            import numpy as _np
            dt = _mybir.dt.np(alloc.dtype)
            out_names.append(name)
            out_avals.append(jax.core.ShapedArray(tuple(alloc.tensor_shape), dt))
            zero_outs.append(_np.zeros(tuple(alloc.tensor_shape), dt))
    n_params = len(in_names)
    n_outs = len(out_avals)
    in_names = in_names + out_names + ([partition_name] if partition_name else [])
    donate = tuple(range(n_params, n_params + n_outs))

    def _body(*args):
        operands = list(args)
        if partition_name is not None:
            operands.append(partition_id_tensor())
        return tuple(_bass_exec_p.bind(
            *operands, out_avals=tuple(out_avals), in_names=tuple(in_names),
            out_names=tuple(out_names), lowering_input_output_aliases=(),
            sim_require_finite=True, sim_require_nnan=True, nc=nc))

    devices = jax.devices()[:n_cores]
    mesh = Mesh(np.asarray(devices), ("core",))
    sharded = jax.jit(
        shard_map(_body, mesh=mesh,
                  in_specs=(PartitionSpec("core"),) * (n_params + n_outs),
                  out_specs=(PartitionSpec("core"),) * n_outs,
                  check_rep=False),
        donate_argnums=donate, keep_unused=True)
    per_core = [[np.asarray(m[nm]) for nm in in_names[:n_params]] for m in in_maps]
    concat_in = [np.concatenate([per_core[c][i] for c in range(n_cores)], axis=0)
                 for i in range(n_params)]
    from jax.sharding import NamedSharding
    shard = NamedSharding(mesh, PartitionSpec("core"))
    dev_in = [jax.device_put(x, shard) for x in concat_in]
    times = []
    out = None
    for it in range(iters + 1):
        zs = [jax.device_put(np.zeros((n_cores * z.shape[0], *z.shape[1:]), z.dtype), shard)
              for z in zero_outs]
        jax.block_until_ready(zs)
        t0 = _time.perf_counter()
        out = sharded(*dev_in, *zs)
        jax.block_until_ready(out)
        t1 = _time.perf_counter()
        times.append(t1 - t0)
    return times, out_names, out
